# revision 3
# baseline (speedup 1.0000x reference)
"""CrossViewEnhancement Trainium2 kernel (8-core batch-parallel SPMD).

Reference computation (per batch element):
    q = avgpool2(conv1x1(bev_x, qw, qb))                   [C8, 64, 64]
    k = avgpool2(conv1x1(front_x, kw, kb)).mean(h)         [C8, 64]
    v = avgpool2(conv3x3(front_x, vw, vb, pad=1))          [C, 64, 64]
    e = einsum(k, q); L2-normalize over h per column       [64, 64]
    T = e * v.sum(h); nn-upsample x2                       [C, 128, 128]
    out = cat(bev[:16], conv3x3(cat(bev[16:], T), fw, fb))

Algebraic restructuring (validated exactly vs reference in fp32 numpy):
  * v only enters via vsum = v.sum(h): the 3x3 conv collapses to 1-D 3-tap
    convs over colsum(front_x) with row-0 / row-127 edge corrections
    (19.3 GFLOP -> 0.15 GFLOP).
  * k only needs colsum; q's 1x1 conv commutes with pooling - the 2x2
    pooling is folded into strided matmul rhs access patterns (4
    accumulating matmuls) reading the bev halo bands.
  * conv3x3 over the x2-nearest-upsampled T decomposes into 4 output-parity
    phases, each a 2x2-tap conv on half-res Tp with parity-summed weights.
  * the dense bev-channel part of the fusion conv is 9 shifted matmuls
    over zero-padded bf16 halo bands covering all 256 bev channels
    (fusion weights zero-padded over channels 0..15 - identical PE cost).

Performance structure (v2): the Tile toolchain splits every matmul into a
standalone LDWEIGHTS + a non-self-loading MATMUL; on TRN2 hardware the
PE-array weight load (~112 ns) serializes with the matmul stream, and the
baseline spent ~198 us of a 578 us kernel on weight loads.  All heavy
loops are therefore ordered weight-stationary: each stationary weight is
streamed against several PSUM banks (Part A: the 4 row-quarters of a row
group; Part B / q: 4 row groups at once), and `_dedup_ldweights` deletes
the now-redundant LDWEIGHTS instructions post-Tile (they carry no
semaphore updates; vestigial waits are folded into the next matmul).
Part B stages its half-res output for 4 row groups in bf16 SBUF.

Host-side prep is constant folding + dtype/layout only: weight transposes
and parity-sums in numpy, plus bf16 copies of the two activation inputs
(they are consumed in bf16 by the tensor engine anyway).

Toolchain constraints honored throughout: the DMA DIRECT2D instruction
encodes at most ONE semaphore wait, so every DMA writes a FRESH tile
(no slot reuse); recycled pool address windows are separated by
all-engine barriers; multi-dependency PSUM->SBUF copies run on the
vector engine (larger wait budget than ACT).
"""

import numpy as np
import ml_dtypes

import concourse.bass as bass
import concourse.mybir as mybir
from concourse.alu_op_type import AluOpType
from concourse.tile import TileContext
from concourse.bass_utils import run_bass_kernel_spmd

F32 = mybir.dt.float32
BF16 = mybir.dt.bfloat16
AX = mybir.AxisListType
AF = mybir.ActivationFunctionType

B, C, H, W = 8, 256, 128, 128
C8 = 32
CO = C - 16          # 240
HP = H // 2          # 64
WP = W // 2          # 64
NCORES = 8
HB = 130             # halo row length (128 + 2)
TPW = WP + 2         # 66
NB = 8               # output row groups of 16
MBLK = [(0, 128), (128, 112)]    # out-channel blocks of the 240
KBLK = [(0, 128), (128, 128)]    # input-channel blocks of 256


def _tap_groups(par):
    return [[0], [1, 2]] if par == 0 else [[0, 1], [2]]


def _tap_offsets(par):
    return [-1, 0] if par == 0 else [0, 1]


def _prep_inputs(inputs):
    bf = ml_dtypes.bfloat16
    qw = np.asarray(inputs["qw"], np.float32)[:, :, 0, 0]
    kw = np.asarray(inputs["kw"], np.float32)[:, :, 0, 0]
    vw = np.asarray(inputs["vw"], np.float32)
    vb = np.asarray(inputs["vb"], np.float32)
    qb = np.asarray(inputs["qb"], np.float32)
    kb = np.asarray(inputs["kb"], np.float32)
    fw = np.asarray(inputs["fw"], np.float32)
    fb = np.asarray(inputs["fb"], np.float32)

    W2 = vw.sum(axis=2)                               # [C, C, 3]
    WV = np.zeros((9, C, C), np.float32)              # [src*3+dx, cin, cout]
    for dx in range(3):
        WV[0 * 3 + dx] = W2[:, :, dx].T
        WV[1 * 3 + dx] = -vw[:, :, 0, dx].T           # -row127 correction
        WV[2 * 3 + dx] = -vw[:, :, 2, dx].T           # -row0 correction
    Wq = qw.T * 0.25                                  # [C, C8]
    Wk = kw.T / 256.0                                 # [C, C8]
    # Part A weights over all 256 bev channels, rows 0..15 zero.
    WA = np.zeros((9, C, CO), np.float32)             # [dy*3+dx, cin, o]
    fwA = np.transpose(fw[:, :CO], (2, 3, 1, 0))      # [dy, dx, cin240, o]
    WA[:, 16:, :] = fwA.reshape(9, CO, CO)
    fwB = fw[:, CO:]                                  # [240, 256, 3, 3]
    WB = np.zeros((16, C, CO), np.float32)            # [((ph*2+pw)*2+i)*2+j]
    for ph in range(2):
        for pw in range(2):
            for i, dys in enumerate(_tap_groups(ph)):
                for j, dxs in enumerate(_tap_groups(pw)):
                    acc = np.zeros((C, CO), np.float32)
                    for dy in dys:
                        for dx in dxs:
                            acc += fwB[:, :, dy, dx].T
                    WB[((ph * 2 + pw) * 2 + i) * 2 + j] = acc
    front = np.asarray(inputs["front_x"], np.float32)
    bev = np.asarray(inputs["bev_x"], np.float32)
    shared = {
        "WV": WV.astype(bf),
        "Wq": Wq.astype(bf),
        "Wk": Wk.astype(bf),
        "WA": WA.astype(bf),
        "WB": WB.astype(bf),
        "vbias": (64.0 * vb).astype(np.float32),
        "qb": qb.astype(np.float32),
        "kb": kb.astype(np.float32),
        "fb": fb.astype(np.float32),
        "ones": np.ones((C8, 128), bf),
    }
    in_maps = []
    for b in range(NCORES):
        m = {
            "front_b": np.ascontiguousarray(front[b].astype(bf)),
            "bev_b": np.ascontiguousarray(bev[b].astype(bf)),
            "bev16": np.ascontiguousarray(bev[b, :16]),
        }
        m.update(shared)
        in_maps.append(m)
    return in_maps


def _build_module():
    nc = bass.Bass()
    fx_d = nc.dram_tensor("front_b", [C, H, W], BF16, kind="ExternalInput")
    bx_d = nc.dram_tensor("bev_b", [C, H, W], BF16, kind="ExternalInput")
    b16_d = nc.dram_tensor("bev16", [16, H, W], F32, kind="ExternalInput")
    WV_d = nc.dram_tensor("WV", [9, C, C], BF16, kind="ExternalInput")
    Wq_d = nc.dram_tensor("Wq", [C, C8], BF16, kind="ExternalInput")
    Wk_d = nc.dram_tensor("Wk", [C, C8], BF16, kind="ExternalInput")
    WA_d = nc.dram_tensor("WA", [9, C, CO], BF16, kind="ExternalInput")
    WB_d = nc.dram_tensor("WB", [16, C, CO], BF16, kind="ExternalInput")
    vbias_d = nc.dram_tensor("vbias", [C], F32, kind="ExternalInput")
    qb_d = nc.dram_tensor("qb", [C8], F32, kind="ExternalInput")
    kb_d = nc.dram_tensor("kb", [C8], F32, kind="ExternalInput")
    fb_d = nc.dram_tensor("fb", [CO], F32, kind="ExternalInput")
    ones_d = nc.dram_tensor("ones", [C8, 128], BF16, kind="ExternalInput")
    out_d = nc.dram_tensor("out", [C, H, W], F32, kind="ExternalOutput")

    with TileContext(nc) as tc:
        # out[:16] = bev[:16] straight through, DRAM->DRAM
        nc.sync.dma_start(out=out_d[0:16], in_=b16_d[:])

        with (
            tc.tile_pool(name="stage", bufs=1) as stp,
            tc.tile_pool(name="early", bufs=1) as epp,
            tc.tile_pool(name="bands", bufs=1) as bandp,
        ):
            # --------- bev halo bands (bf16, all 256 channels) -----------
            # band[g][blk]: padded rows 16g..16g+17 (h = 16g-1..16g+16)
            bands = [[None, None] for _ in range(NB)]
            for g in range(NB):
                for bl, (c0, cs) in enumerate(KBLK):
                    bt = bandp.tile([cs, 18 * HB], BF16,
                                    name=f"band_{g}_{bl}",
                                    tag=f"band_{g}_{bl}")
                    bands[g][bl] = bt
                    v = bt[:].rearrange("p (r c) -> p r c", r=18)
                    nc.gpsimd.memset(v[:, :, 0:1], 0.0)
                    nc.gpsimd.memset(v[:, :, HB - 1:HB], 0.0)
                    h_lo, r0, nrows = 16 * g - 1, 0, 18
                    if g == 0:
                        nc.gpsimd.memset(v[:, 0:1, :], 0.0)
                        h_lo, r0, nrows = 0, 1, 17
                    if g == NB - 1:
                        nc.gpsimd.memset(v[:, 17:18, :], 0.0)
                        nrows -= 1
                    nc.sync.dma_start(
                        out=v[:, r0:r0 + nrows, 1:1 + W],
                        in_=bx_d[c0:c0 + cs, h_lo:h_lo + nrows, :])

            # --------- small weights + colsum inputs (early pool) --------
            WV_t = []
            for sd in range(9):
                row = []
                for kb_i, (k0, ks) in enumerate(KBLK):
                    t = epp.tile([ks, C], BF16, name=f"WV_{sd}_{kb_i}",
                                 tag=f"WV_{sd}_{kb_i}")
                    nc.sync.dma_start(out=t[:], in_=WV_d[sd, k0:k0 + ks, :])
                    row.append(t)
                WV_t.append(row)
            Wq_t, Wk_t = [], []
            for kb_i, (k0, ks) in enumerate(KBLK):
                tq = epp.tile([ks, C8], BF16, name=f"Wq_{kb_i}",
                              tag=f"Wq_{kb_i}")
                nc.sync.dma_start(out=tq[:], in_=Wq_d[k0:k0 + ks, :])
                Wq_t.append(tq)
                tk = epp.tile([ks, C8], BF16, name=f"Wk_{kb_i}",
                              tag=f"Wk_{kb_i}")
                nc.sync.dma_start(out=tk[:], in_=Wk_d[k0:k0 + ks, :])
                Wk_t.append(tk)
            ones_t = epp.tile([C8, 128], BF16, name="ones_t", tag="ones_t")
            nc.sync.dma_start(out=ones_t[:], in_=ones_d[:])
            vbias_t = []
            for bl in range(2):
                t = epp.tile([128, 1], F32, name=f"vbias_{bl}",
                             tag=f"vbias_{bl}")
                nc.sync.dma_start(
                    out=t[:], in_=vbias_d[bl * 128:(bl + 1) * 128].unsqueeze(1))
                vbias_t.append(t)
            qb_t = epp.tile([C8, 1], F32, name="qb_t", tag="qb_t")
            nc.sync.dma_start(out=qb_t[:], in_=qb_d[:].unsqueeze(1))
            kb_t = epp.tile([C8, 1], F32, name="kb_t", tag="kb_t")
            nc.sync.dma_start(out=kb_t[:], in_=kb_d[:].unsqueeze(1))

            # ---- stage 1: colsum + edge rows of front (scoped pool) ----
            X3b, P2b = [], []
            with tc.tile_pool(name="fstr", bufs=1) as fp_:
                for bl in range(2):
                    c0 = bl * 128
                    csum = epp.tile([128, W], F32, name=f"colsum_{bl}",
                                    tag=f"colsum_{bl}")
                    for half in range(2):
                        ch = fp_.tile([128, 64, W], BF16,
                                      name=f"fch_{bl}_{half}",
                                      tag=f"fch_{bl}_{half}")
                        nc.sync.dma_start(
                            out=ch[:],
                            in_=fx_d[c0:c0 + 128,
                                     half * 64:(half + 1) * 64, :])
                        if half == 0:
                            nc.vector.tensor_reduce(
                                out=csum[:],
                                in_=ch[:].rearrange("p r c -> p c r"),
                                axis=AX.X, op=AluOpType.add)
                        else:
                            part = fp_.tile([128, W], F32,
                                            name=f"fpart_{bl}",
                                            tag=f"fpart_{bl}")
                            nc.vector.tensor_reduce(
                                out=part[:],
                                in_=ch[:].rearrange("p r c -> p c r"),
                                axis=AX.X, op=AluOpType.add)
                            nc.vector.tensor_add(
                                out=csum[:], in0=csum[:], in1=part[:])
                    r0t = epp.tile([128, W], BF16, name=f"r0_{bl}",
                                   tag=f"r0_{bl}")
                    rLt = epp.tile([128, W], BF16, name=f"rL_{bl}",
                                   tag=f"rL_{bl}")
                    nc.sync.dma_start(out=r0t[:], in_=fx_d[c0:c0 + 128, 0, :])
                    nc.sync.dma_start(out=rLt[:],
                                      in_=fx_d[c0:c0 + 128, H - 1, :])
                    x3v = epp.tile([128, 3 * HB], BF16, name=f"x3_{bl}",
                                   tag=f"x3_{bl}")
                    xv = x3v[:].rearrange("p (s c) -> p s c", s=3)
                    nc.gpsimd.memset(xv[:, :, 0:1], 0.0)
                    nc.gpsimd.memset(xv[:, :, HB - 1:HB], 0.0)
                    nc.vector.tensor_copy(out=xv[:, 0, 1:1 + W], in_=csum[:])
                    nc.vector.tensor_copy(out=xv[:, 1, 1:1 + W], in_=rLt[:])
                    nc.vector.tensor_copy(out=xv[:, 2, 1:1 + W], in_=r0t[:])
                    X3b.append(xv)
                    p2 = epp.tile([128, WP], BF16, name=f"p2_{bl}",
                                  tag=f"p2_{bl}")
                    cs3 = csum[:].rearrange("p (w two) -> p w two", two=2)
                    nc.vector.tensor_tensor(
                        out=p2[:], in0=cs3[:, :, 0], in1=cs3[:, :, 1],
                        op=AluOpType.add)
                    P2b.append(p2)

            tc.strict_bb_all_engine_barrier()

            # --------- heavy weights (recycle the colsum window) ---------
            with tc.tile_pool(name="wpers", bufs=1) as wpp:
                WA_t = []
                for sd in range(9):
                    row = []
                    for kb_i, (k0, ks) in enumerate(KBLK):
                        t = wpp.tile([ks, CO], BF16, name=f"WA_{sd}_{kb_i}",
                                     tag=f"WA_{sd}_{kb_i}")
                        nc.sync.dma_start(out=t[:], in_=WA_d[sd, k0:k0 + ks, :])
                        row.append(t)
                    WA_t.append(row)
                WB_t = []
                for cc in range(16):
                    row = []
                    for kb_i, (k0, ks) in enumerate(KBLK):
                        t = wpp.tile([ks, CO], BF16, name=f"WB_{cc}_{kb_i}",
                                     tag=f"WB_{cc}_{kb_i}")
                        nc.sync.dma_start(out=t[:], in_=WB_d[cc, k0:k0 + ks, :])
                        row.append(t)
                    WB_t.append(row)
                fb_t = []
                for mb_i, (m0, ms) in enumerate(MBLK):
                    t = wpp.tile([ms, 1], F32, name=f"fb_{mb_i}",
                                 tag=f"fb_{mb_i}")
                    nc.sync.dma_start(out=t[:],
                                      in_=fb_d[m0:m0 + ms].unsqueeze(1))
                    fb_t.append(t)

                # ======== prefix compute: vsum / k / q / e / Tp ==========
                with (
                    tc.tile_pool(name="pref", bufs=1) as prp,
                    tc.tile_pool(name="psp", bufs=1, space="PSUM") as psp,
                ):
                    # ---- stage 2: S -> vsum ----
                    vsum_t = []
                    for mb in range(2):
                        ps = psp.tile([128, W], F32, name="psS", tag="psS")
                        first = True
                        for sd in range(9):
                            src, dx = divmod(sd, 3)
                            for kb_i in range(2):
                                nc.tensor.matmul(
                                    ps[:],
                                    WV_t[sd][kb_i][:, mb * 128:(mb + 1) * 128],
                                    X3b[kb_i][:, src, dx:dx + W],
                                    start=first, stop=(sd == 8 and kb_i == 1))
                                first = False
                        ssb = prp.tile([128, W], F32, name=f"ssb_{mb}",
                                       tag=f"ssb_{mb}")
                        nc.scalar.activation(out=ssb[:], in_=ps[:],
                                             func=AF.Copy, scale=0.25)
                        se = ssb[:].rearrange("p (w two) -> p w two", two=2)
                        vs = prp.tile([128, WP], F32, name=f"vsum_{mb}",
                                      tag=f"vsum_{mb}")
                        nc.vector.scalar_tensor_tensor(
                            out=vs[:], in0=se[:, :, 0],
                            scalar=vbias_t[mb][:], in1=se[:, :, 1],
                            op0=AluOpType.add, op1=AluOpType.add)
                        vsum_t.append(vs)

                    # ---- stage 3: k ----
                    psk = psp.tile([C8, WP], F32, name="psK", tag="psK")
                    nc.tensor.matmul(psk[:], Wk_t[0][:], P2b[0][:],
                                     start=True, stop=False)
                    nc.tensor.matmul(psk[:], Wk_t[1][:], P2b[1][:],
                                     start=False, stop=True)
                    k_t = prp.tile([C8, WP], F32, name="k_t", tag="k_t")
                    nc.vector.tensor_scalar_add(out=k_t[:], in0=psk[:],
                                                scalar1=kb_t[:])

                    # ---- stage 4: q (pooling inside strided rhs) + qk ----
                    # weight-stationary: Wq[kb] streams a pair of row
                    # groups x 4 pooling taps per load (redundant
                    # LDWEIGHTS dedup'd post-Tile).
                    qk_t = prp.tile([C8, HP * WP], BF16, name="qk_t",
                                    tag="qk_t")
                    qkv = qk_t[:].rearrange("p (h w) -> p h w", w=WP)
                    for pr in range(4):
                        psqs = [psp.tile([C8, 8 * WP], F32, name="psQ",
                                         tag="psQ", bufs=4)
                                for _ in range(2)]
                        for kb_i in range(2):
                            for t in range(2):
                                g = 2 * pr + t
                                bv = bands[g][kb_i][:].rearrange(
                                    "p (r c) -> p r c", c=HB)
                                for i in range(2):
                                    for j in range(2):
                                        rhs = bv[:, 1 + i:17 + i:2,
                                                 1 + j:129 + j:2]
                                        nc.tensor.matmul(
                                            psqs[t][:], Wq_t[kb_i][:], rhs,
                                            start=(kb_i == 0 and i == 0
                                                   and j == 0),
                                            stop=(kb_i == 1 and i == 1
                                                  and j == 1),
                                            skip_group_check=True)
                        for t in range(2):
                            g = 2 * pr + t
                            qtmp = prp.tile([C8, 8 * WP], F32, name="qtmp",
                                            tag="qtmp", bufs=2)
                            nc.vector.tensor_scalar_add(out=qtmp[:],
                                                        in0=psqs[t][:],
                                                        scalar1=qb_t[:])
                            kv = k_t[:].unsqueeze(1).broadcast_to([C8, 8, WP])
                            nc.vector.tensor_tensor(
                                out=qkv[:, g * 8:(g + 1) * 8, :],
                                in0=qtmp[:].rearrange("p (h w) -> p h w",
                                                      w=WP),
                                in1=kv, op=AluOpType.mult)

                    # ---- stage 5: e (replicated) + column norms ----
                    e_t = prp.tile([128, HP * WP], BF16, name="e_t",
                                   tag="e_t")
                    for chn in range(8):
                        nsl = slice(chn * 512, (chn + 1) * 512)
                        pse = psp.tile([128, 512], F32, name="psE", tag="psE",
                                       bufs=2)
                        nc.tensor.matmul(pse[:], ones_t[:], qk_t[:, nsl],
                                         start=True, stop=True)
                        nc.vector.tensor_copy(out=e_t[:, nsl], in_=pse[:])
                    n2 = prp.tile([128, WP], F32, name="n2", tag="n2")
                    for chn in range(8):
                        scr = prp.tile([128, 8 * WP], F32, name="scr",
                                       tag="scr", bufs=2)
                        esl = e_t[:, chn * 8 * WP:(chn + 1) * 8 * WP]
                        nc.vector.tensor_tensor(out=scr[:], in0=esl, in1=esl,
                                                op=AluOpType.mult)
                        part = prp.tile([128, WP], F32, name="npart",
                                        tag="npart", bufs=2)
                        nc.vector.tensor_reduce(
                            out=part[:],
                            in_=scr[:].rearrange("p (h w) -> p w h", w=WP),
                            axis=AX.X, op=AluOpType.add)
                        if chn == 0:
                            nc.vector.tensor_copy(out=n2[:], in_=part[:])
                        else:
                            nc.vector.tensor_add(out=n2[:], in0=n2[:],
                                                 in1=part[:])
                    nrm = prp.tile([128, WP], F32, name="nrm", tag="nrm")
                    nc.scalar.sqrt(out=nrm[:], in_=n2[:])
                    rinv = prp.tile([128, WP], F32, name="rinv", tag="rinv")
                    nc.vector.reciprocal(out=rinv[:], in_=nrm[:])

                    # ---- stage 6: Tp_pad = (vsum * rinv) x e ----
                    tp_t = []
                    for bl in range(2):
                        vs2 = prp.tile([128, WP], F32, name=f"vs2_{bl}",
                                       tag=f"vs2_{bl}")
                        nc.vector.tensor_tensor(
                            out=vs2[:], in0=vsum_t[bl][:], in1=rinv[:],
                            op=AluOpType.mult)
                        tp = stp.tile([128, (HP + 2) * TPW], BF16,
                                      name=f"tp_{bl}", tag=f"tp_{bl}")
                        tp_t.append(tp)
                        tv = tp[:].rearrange("p (r c) -> p r c", c=TPW)
                        nc.gpsimd.memset(tv[:, 0:1, :], 0.0)
                        nc.gpsimd.memset(tv[:, HP + 1:HP + 2, :], 0.0)
                        nc.gpsimd.memset(tv[:, :, 0:1], 0.0)
                        nc.gpsimd.memset(tv[:, :, TPW - 1:TPW], 0.0)
                        ev = e_t[:].rearrange("p (h w) -> p h w", w=WP)
                        v2 = vs2[:].unsqueeze(1).broadcast_to([128, HP, WP])
                        nc.vector.tensor_tensor(
                            out=tv[:, 1:1 + HP, 1:1 + WP], in0=v2, in1=ev,
                            op=AluOpType.mult)

                tc.strict_bb_all_engine_barrier()

                # ===== main loop: per quad of 4 row groups ==============
                # Part B (all phases, weight-stationary over the quad's 4
                # psum banks) stages into bf16 SBUF; then Part A per group
                # (weight-stationary over the 4 row-quarters) adds the
                # staged Part B and DMAs out.
                with (
                    tc.tile_pool(name="gout", bufs=2) as gop,
                    tc.tile_pool(name="gstg", bufs=1) as gsp,
                    tc.tile_pool(name="psm", bufs=8, space="PSUM") as psm,
                ):
                    for qd in range(2):
                        gs = [4 * qd + t for t in range(4)]
                        stg = [[gsp.tile([ms, 16 * W], BF16,
                                         name=f"stg_{t}_{mb_i}",
                                         tag=f"stg_{t}_{mb_i}")
                                for mb_i, (m0, ms) in enumerate(MBLK)]
                               for t in range(4)]
                        # ---- Part B: phases x quad, weight-stationary ----
                        for mb_i, (m0, ms) in enumerate(MBLK):
                            for ph in range(2):
                                ro = _tap_offsets(ph)
                                for pw in range(2):
                                    co = _tap_offsets(pw)
                                    pbs = [psm.tile([ms, 8 * WP], F32,
                                                    name="psm", tag="psm")
                                           for _ in range(4)]
                                    ki = 0
                                    for i in range(2):
                                        for j in range(2):
                                            cc = ((ph * 2 + pw) * 2 + i) * 2 + j
                                            for kb_i in range(2):
                                                wap = WB_t[cc][kb_i][
                                                    :, m0:m0 + ms]
                                                for t in range(4):
                                                    g = gs[t]
                                                    tv = tp_t[kb_i][:].rearrange(
                                                        "p (r c) -> p r c",
                                                        c=TPW)
                                                    rhs = tv[:,
                                                             8 * g + 1 + ro[i]:
                                                             8 * g + 9 + ro[i],
                                                             1 + co[j]:
                                                             1 + co[j] + WP]
                                                    nc.tensor.matmul(
                                                        pbs[t][:], wap, rhs,
                                                        start=(ki == 0),
                                                        stop=(ki == 7),
                                                        skip_group_check=True)
                                                ki += 1
                                    for t in range(4):
                                        sv = stg[t][mb_i][:].rearrange(
                                            "p (h two w pw2) -> p h two w pw2",
                                            two=2, w=WP, pw2=2)
                                        nc.vector.tensor_scalar_add(
                                            out=sv[:, :, ph, :, pw],
                                            in0=pbs[t][:].rearrange(
                                                "p (h w) -> p h w", w=WP),
                                            scalar1=fb_t[mb_i][:])
                        # ---- Part A: per group, weight-stationary over
                        # the 4 row-quarters ----
                        for t in range(4):
                            g = gs[t]
                            for mb_i, (m0, ms) in enumerate(MBLK):
                                pas = [psm.tile([ms, 4 * W], F32,
                                                name="psm", tag="psm")
                                       for _ in range(4)]
                                ot = gop.tile([ms, 16, W], F32,
                                              name=f"ot{mb_i}",
                                              tag=f"ot{mb_i}")
                                si_ = 0
                                for dy in range(3):
                                    for dx in range(3):
                                        for kb_i in range(2):
                                            wap = WA_t[dy * 3 + dx][kb_i][
                                                :, m0:m0 + ms]
                                            bv = bands[g][kb_i][:].rearrange(
                                                "p (r c) -> p r c", c=HB)
                                            for n in range(4):
                                                rhs = bv[:,
                                                         4 * n + dy:
                                                         4 * n + dy + 4,
                                                         dx:dx + W]
                                                nc.tensor.matmul(
                                                    pas[n][:], wap, rhs,
                                                    start=(si_ == 0),
                                                    stop=(si_ == 17),
                                                    skip_group_check=True)
                                            si_ += 1
                                for n in range(4):
                                    nc.vector.tensor_add(
                                        out=ot[:, n * 4:(n + 1) * 4, :],
                                        in0=pas[n][:].rearrange(
                                            "p (r c) -> p r c", c=W),
                                        in1=stg[t][mb_i][
                                            :, n * 4 * W:(n + 1) * 4 * W]
                                        .rearrange("p (r c) -> p r c", c=W))
                                nc.sync.dma_start(
                                    out=out_d[16 + m0:16 + m0 + ms,
                                              16 * g:16 * (g + 1), :],
                                    in_=ot[:])
    return nc


def _dedup_ldweights(nc):
    """Tile splits every matmul into a standalone InstLdweights plus a
    non-self-loading InstMatmult. Consecutive matmuls reusing the same
    stationary therefore emit redundant PE-array loads (~112 ns each,
    serialized with the matmul stream on TRN2). Delete an InstLdweights
    when the previous one on the PE stream loaded the identical weights
    AP and nothing in between clobbered the array. Ldweights carry no
    semaphore updates; any (vestigial) waits are folded into the next
    matmul, which `_legalize_waits` then splits if needed."""
    n_drop = 0
    PE = mybir.EngineType.PE
    for fn in nc.m.functions:
        for bb in fn.blocks:
            out = []
            last_key = None
            pending_waits = []
            for ins in bb.instructions:
                if isinstance(ins, mybir.InstLdweights):
                    key = (str(ins.ins[0]), str(ins.tile_position),
                           str(ins.tile_size), str(ins.perf_mode),
                           str(ins.is_transpose))
                    if key == last_key:
                        si = ins.sync_info
                        if si is not None and si.on_update:
                            out.append(ins)   # cannot drop: has updates
                            continue
                        if si is not None and si.on_wait:
                            pending_waits.extend(si.on_wait)
                        n_drop += 1
                        continue
                    last_key = key
                    out.append(ins)
                elif isinstance(ins, mybir.InstMatmult):
                    if ins.is_transpose:
                        last_key = None
                    if pending_waits:
                        si = ins.sync_info
                        w = list(si.on_wait) if si else []
                        u = list(si.on_update) if si else []
                        ins.sync_info = mybir.SyncInfo(
                            on_wait=w + pending_waits, on_update=u)
                        pending_waits = []
                    out.append(ins)
                else:
                    if (getattr(ins, "engine", None) == PE
                            and not isinstance(ins,
                                               mybir.InstEventSemaphore)):
                        last_key = None
                    out.append(ins)
            assert not pending_waits, "dangling ldweights waits after dedup"
            bb.instructions[:] = out
    return n_drop


def _legalize_waits(nc):
    """This toolchain's codegen accepts at most ONE semaphore wait per
    instruction (the TPB `events` field has a single wait slot). Tile's
    wait assignment can attach several. Hoist all but one wait onto
    standalone EventSemaphore instructions placed immediately before the
    owner on the same engine stream - strictly stronger synchronization,
    so always safe."""
    n_split = 0
    for fn in nc.m.functions:
        for bb in fn.blocks:
            out = []
            for ins in bb.instructions:
                si = ins.sync_info
                if si is not None and len(si.on_wait) > 1:
                    extra = list(si.on_wait[:-1])
                    keep = si.on_wait[-1]
                    for idx, wt in enumerate(extra):
                        ev = mybir.InstEventSemaphore(
                            name=f"{ins.name}_hw{idx}",
                            engine=ins.engine,
                            sync_info=mybir.SyncInfo(on_wait=[wt],
                                                     on_update=[]),
                        )
                        out.append(ev)
                        n_split += 1
                    ins.sync_info = mybir.SyncInfo(
                        on_wait=[keep], on_update=list(si.on_update))
                out.append(ins)
            bb.instructions[:] = out
    return n_split


_NC_CACHE = None


def kernel(**inputs):
    global _NC_CACHE
    in_maps = _prep_inputs(inputs)
    if _NC_CACHE is None:
        _NC_CACHE = _build_module()
        _dedup_ldweights(_NC_CACHE)
        _legalize_waits(_NC_CACHE)
    res = run_bass_kernel_spmd(_NC_CACHE, in_maps, list(range(NCORES)))
    out = np.stack([res.results[b]["out"] for b in range(NCORES)], axis=0)
    return out.astype(np.float32)


# revision 13
# speedup vs baseline: 1.2621x; 1.2621x over previous
"""CrossViewEnhancement Trainium2 kernel (8-core batch-parallel SPMD).

Reference computation (per batch element):
    q = avgpool2(conv1x1(bev_x, qw, qb))                   [C8, 64, 64]
    k = avgpool2(conv1x1(front_x, kw, kb)).mean(h)         [C8, 64]
    v = avgpool2(conv3x3(front_x, vw, vb, pad=1))          [C, 64, 64]
    e = einsum(k, q); L2-normalize over h per column       [64, 64]
    T = e * v.sum(h); nn-upsample x2                       [C, 128, 128]
    out = cat(bev[:16], conv3x3(cat(bev[16:], T), fw, fb))

Algebraic restructuring (validated exactly vs reference in fp32 numpy):
  * v only enters via vsum = v.sum(h): the 3x3 conv collapses to 1-D 3-tap
    convs over colsum(front_x) with row-0 / row-127 edge corrections.
  * k only needs colsum; q's 1x1 conv commutes with pooling - the 2x2
    pooling is folded into strided matmul rhs access patterns reading the
    bev halo bands.
  * conv3x3 over the x2-nearest-upsampled T decomposes into 4 output-parity
    phases, each a 2x2-tap conv on half-res Tp with parity-summed weights.
  * the dense bev-channel part of the fusion conv (Part A) is 9 shifted
    matmuls over zero-padded bf16 halo bands covering all 256 bev channels.

Schedule (v3): the profiled v1/v2 kernels spent ~200 us of a ~575 us span
in a serial prefix (input DMA + DVE column-sum + q/e/Tp) with the PE
idle, then ~370 us PE-saturated in the fused-conv main loop. v3 hides the
prefix behind Part-A work, which depends only on the bev bands + WA:
    A(g0) | vsum k q | A(g1).mb0 | e | A(g1).mb1 | B(g0) B(g1)
          | A(g2) B(g2) | ... | A(g7) B(g7)
Part A drains psum straight into the per-group output tile (ACT-engine
copies); Part B scatter-adds its 4 phase outputs (+bias) into the same
tile, which then DMAs out. The column-sum is chunked (8 rows per DMA)
and reduced as chunks land, split across the DVE and Pool engines.
DMA emission order prioritizes the prefix critical path: WA + first
bands -> front chunks -> small weights -> remaining bands -> WB.
No all-engine barriers: no SBUF window is recycled across pools (the
PSUM prefix->B pool handoff is ordered transitively through the Tp
dependency chain).

A post-Tile `_dedup_ldweights` pass drops PE weight reloads for
consecutive same-stationary matmuls (q taps, e's replicated-ones), and
`_legalize_waits` splits multi-wait instructions (the TPB encoding has
one wait slot).
"""

import numpy as np
import ml_dtypes

import concourse.bass as bass
import concourse.mybir as mybir
from concourse.alu_op_type import AluOpType
from concourse.tile import TileContext
from concourse.bass_utils import run_bass_kernel_spmd

F32 = mybir.dt.float32
BF16 = mybir.dt.bfloat16
AX = mybir.AxisListType
AF = mybir.ActivationFunctionType

B, C, H, W = 8, 256, 128, 128
C8 = 32
CO = C - 16          # 240
HP = H // 2          # 64
WP = W // 2          # 64
NCORES = 8
HB = 130             # halo row length (128 + 2)
TPW = WP + 2         # 66
NB = 8               # output row groups of 16
MBLK = [(0, 128), (128, 112)]    # out-channel blocks of the 240
KBLK = [(0, 128), (128, 128)]    # input-channel blocks of 256
FCH = 8              # front colsum chunk rows


def _tap_groups(par):
    return [[0], [1, 2]] if par == 0 else [[0, 1], [2]]


def _tap_offsets(par):
    return [-1, 0] if par == 0 else [0, 1]


def _prep_inputs(inputs):
    bf = ml_dtypes.bfloat16
    qw = np.asarray(inputs["qw"], np.float32)[:, :, 0, 0]
    kw = np.asarray(inputs["kw"], np.float32)[:, :, 0, 0]
    vw = np.asarray(inputs["vw"], np.float32)
    vb = np.asarray(inputs["vb"], np.float32)
    qb = np.asarray(inputs["qb"], np.float32)
    kb = np.asarray(inputs["kb"], np.float32)
    fw = np.asarray(inputs["fw"], np.float32)
    fb = np.asarray(inputs["fb"], np.float32)

    W2 = vw.sum(axis=2)                               # [C, C, 3]
    WV = np.zeros((9, C, C), np.float32)              # [src*3+dx, cin, cout]
    for dx in range(3):
        WV[0 * 3 + dx] = W2[:, :, dx].T
        WV[1 * 3 + dx] = -vw[:, :, 0, dx].T           # -row127 correction
        WV[2 * 3 + dx] = -vw[:, :, 2, dx].T           # -row0 correction
    Wq = qw.T * 0.25                                  # [C, C8]
    Wk = kw.T / 256.0                                 # [C, C8]
    # Part A weights over all 256 bev channels, rows 0..15 zero.
    WA = np.zeros((9, C, CO), np.float32)             # [dy*3+dx, cin, o]
    fwA = np.transpose(fw[:, :CO], (2, 3, 1, 0))      # [dy, dx, cin240, o]
    WA[:, 16:, :] = fwA.reshape(9, CO, CO)
    fwB = fw[:, CO:]                                  # [240, 256, 3, 3]
    WB = np.zeros((16, C, CO), np.float32)            # [((ph*2+pw)*2+i)*2+j]
    for ph in range(2):
        for pw in range(2):
            for i, dys in enumerate(_tap_groups(ph)):
                for j, dxs in enumerate(_tap_groups(pw)):
                    acc = np.zeros((C, CO), np.float32)
                    for dy in dys:
                        for dx in dxs:
                            acc += fwB[:, :, dy, dx].T
                    WB[((ph * 2 + pw) * 2 + i) * 2 + j] = acc
    front = np.asarray(inputs["front_x"], np.float32)
    bev = np.asarray(inputs["bev_x"], np.float32)
    shared = {
        "WV": WV.astype(bf),
        "Wq": Wq.astype(bf),
        "Wk": Wk.astype(bf),
        "WA": WA.astype(bf),
        "WB": WB.astype(bf),
        "vbias": (64.0 * vb).astype(np.float32),
        "qb": qb.astype(np.float32),
        "kb": kb.astype(np.float32),
        "fb": fb.astype(np.float32),
        "ones": np.ones((C8, 128), bf),
    }
    in_maps = []
    for b in range(NCORES):
        m = {
            "front_b": np.ascontiguousarray(front[b].astype(bf)),
            "bev_b": np.ascontiguousarray(bev[b].astype(bf)),
            "bev16": np.ascontiguousarray(bev[b, :16]),
        }
        m.update(shared)
        in_maps.append(m)
    return in_maps


def _build_module():
    nc = bass.Bass()
    fx_d = nc.dram_tensor("front_b", [C, H, W], BF16, kind="ExternalInput")
    bx_d = nc.dram_tensor("bev_b", [C, H, W], BF16, kind="ExternalInput")
    b16_d = nc.dram_tensor("bev16", [16, H, W], F32, kind="ExternalInput")
    WV_d = nc.dram_tensor("WV", [9, C, C], BF16, kind="ExternalInput")
    Wq_d = nc.dram_tensor("Wq", [C, C8], BF16, kind="ExternalInput")
    Wk_d = nc.dram_tensor("Wk", [C, C8], BF16, kind="ExternalInput")
    WA_d = nc.dram_tensor("WA", [9, C, CO], BF16, kind="ExternalInput")
    WB_d = nc.dram_tensor("WB", [16, C, CO], BF16, kind="ExternalInput")
    vbias_d = nc.dram_tensor("vbias", [C], F32, kind="ExternalInput")
    qb_d = nc.dram_tensor("qb", [C8], F32, kind="ExternalInput")
    kb_d = nc.dram_tensor("kb", [C8], F32, kind="ExternalInput")
    fb_d = nc.dram_tensor("fb", [CO], F32, kind="ExternalInput")
    ones_d = nc.dram_tensor("ones", [C8, 128], BF16, kind="ExternalInput")
    out_d = nc.dram_tensor("out", [C, H, W], F32, kind="ExternalOutput")

    with TileContext(nc) as tc:
        with (
            tc.tile_pool(name="weights", bufs=1) as wp,
            tc.tile_pool(name="bands", bufs=1) as bandp,
            tc.tile_pool(name="front", bufs=1) as frp,
            tc.tile_pool(name="pref", bufs=1) as prp,
            tc.tile_pool(name="stage", bufs=1) as stp,
            tc.tile_pool(name="gout", bufs=1) as gop,
            tc.tile_pool(name="psa", bufs=3, space="PSUM") as psa,
        ):
            # ============ DMA priority 1: WA + fb + bands g0/g1 =========
            WA_t = []
            for sd in range(9):
                row = []
                for kb_i, (k0, ks) in enumerate(KBLK):
                    t = wp.tile([ks, CO], BF16, name=f"WA_{sd}_{kb_i}",
                                tag=f"WA_{sd}_{kb_i}")
                    nc.sync.dma_start(out=t[:], in_=WA_d[sd, k0:k0 + ks, :])
                    row.append(t)
                WA_t.append(row)
            fb_t = []
            for mb_i, (m0, ms) in enumerate(MBLK):
                t = wp.tile([ms, 1], F32, name=f"fb_{mb_i}", tag=f"fb_{mb_i}")
                nc.sync.dma_start(out=t[:], in_=fb_d[m0:m0 + ms].unsqueeze(1))
                fb_t.append(t)

            bands = [[None, None] for _ in range(NB)]

            def load_band(g):
                for bl, (c0, cs) in enumerate(KBLK):
                    bt = bandp.tile([cs, 18 * HB], BF16,
                                    name=f"band_{g}_{bl}",
                                    tag=f"band_{g}_{bl}")
                    bands[g][bl] = bt
                    v = bt[:].rearrange("p (r c) -> p r c", r=18)
                    nc.gpsimd.memset(v[:, :, 0:1], 0.0)
                    nc.gpsimd.memset(v[:, :, HB - 1:HB], 0.0)
                    h_lo, r0, nrows = 16 * g - 1, 0, 18
                    if g == 0:
                        nc.gpsimd.memset(v[:, 0:1, :], 0.0)
                        h_lo, r0, nrows = 0, 1, 17
                    if g == NB - 1:
                        nc.gpsimd.memset(v[:, 17:18, :], 0.0)
                        nrows -= 1
                    nc.sync.dma_start(
                        out=v[:, r0:r0 + nrows, 1:1 + W],
                        in_=bx_d[c0:c0 + cs, h_lo:h_lo + nrows, :])

            load_band(0)
            load_band(1)

            # ======= DMA priority 2: front chunks + colsum as they land
            # bl=0 reduces on the DVE (transposed tensor_reduce), bl=1 on
            # the Pool engine (contiguous halving tree: it has no free-axis
            # reduce), so the two blocks' column sums proceed in parallel.
            csum_t, X3b, P2b = [], [], []
            NCH = H // FCH
            for bl in range(2):
                c0 = bl * 128
                csum = frp.tile([128, W], F32, name=f"colsum_{bl}",
                                tag=f"colsum_{bl}")
                csum_t.append(csum)
                for chunk in range(NCH):
                    ch = frp.tile([128, FCH * W], BF16,
                                  name=f"fch_{bl}_{chunk}",
                                  tag=f"fch_{bl}", bufs=2)
                    nc.sync.dma_start(
                        out=ch[:],
                        in_=fx_d[c0:c0 + 128,
                                 chunk * FCH:(chunk + 1) * FCH, :])
                    if bl == 0:
                        chv = ch[:].rearrange("p (r c) -> p c r", r=FCH)
                        part = frp.tile([128, W], F32, name="part_0",
                                        tag="part_0", bufs=2)
                        nc.vector.tensor_reduce(out=part[:], in_=chv,
                                                axis=AX.X, op=AluOpType.add)
                        if chunk == 0:
                            nc.vector.tensor_copy(out=csum[:], in_=part[:])
                        else:
                            nc.vector.tensor_add(out=csum[:], in0=csum[:],
                                                 in1=part[:])
                    else:
                        half = FCH * W // 2
                        t1 = frp.tile([128, half], BF16, name="tr1",
                                      tag="tr1")
                        nc.gpsimd.tensor_tensor(
                            out=t1[:], in0=ch[:, 0:half],
                            in1=ch[:, half:2 * half], op=AluOpType.add)
                        lvl = t1
                        sz = half
                        d = 0
                        while sz > W:
                            sz //= 2
                            nxt = frp.tile([128, sz],
                                           BF16 if sz > W else F32,
                                           name=f"tr{d + 2}",
                                           tag=f"tr{d + 2}")
                            nc.gpsimd.tensor_tensor(
                                out=nxt[:], in0=lvl[:, 0:sz],
                                in1=lvl[:, sz:2 * sz], op=AluOpType.add)
                            lvl = nxt
                            d += 1
                        if chunk == 0:
                            nc.gpsimd.tensor_copy(out=csum[:], in_=lvl[:])
                        else:
                            nc.gpsimd.tensor_add(out=csum[:], in0=csum[:],
                                                 in1=lvl[:])

            # edge rows + x3 / p2 assembly (vector; small)
            for bl in range(2):
                c0 = bl * 128
                eng = nc.vector if bl == 0 else nc.gpsimd
                csum = csum_t[bl]
                r0t = frp.tile([128, W], BF16, name=f"r0_{bl}",
                               tag=f"r0_{bl}")
                rLt = frp.tile([128, W], BF16, name=f"rL_{bl}",
                               tag=f"rL_{bl}")
                nc.sync.dma_start(out=r0t[:], in_=fx_d[c0:c0 + 128, 0, :])
                nc.sync.dma_start(out=rLt[:], in_=fx_d[c0:c0 + 128, H - 1, :])
                x3v = frp.tile([128, 3 * HB], BF16, name=f"x3_{bl}",
                               tag=f"x3_{bl}")
                xv = x3v[:].rearrange("p (s c) -> p s c", s=3)
                nc.gpsimd.memset(xv[:, :, 0:1], 0.0)
                nc.gpsimd.memset(xv[:, :, HB - 1:HB], 0.0)
                eng.tensor_copy(out=xv[:, 0, 1:1 + W], in_=csum[:])
                eng.tensor_copy(out=xv[:, 1, 1:1 + W], in_=rLt[:])
                eng.tensor_copy(out=xv[:, 2, 1:1 + W], in_=r0t[:])
                X3b.append(xv)
                p2 = frp.tile([128, WP], BF16, name=f"p2_{bl}",
                              tag=f"p2_{bl}")
                cs3 = csum[:].rearrange("p (w two) -> p w two", two=2)
                eng.tensor_tensor(out=p2[:], in0=cs3[:, :, 0],
                                  in1=cs3[:, :, 1], op=AluOpType.add)
                P2b.append(p2)

            # ========= DMA priority 3: small weights, rest of bands =====
            WV_t = []
            for sd in range(9):
                row = []
                for kb_i, (k0, ks) in enumerate(KBLK):
                    t = wp.tile([ks, C], BF16, name=f"WV_{sd}_{kb_i}",
                                tag=f"WV_{sd}_{kb_i}")
                    nc.sync.dma_start(out=t[:], in_=WV_d[sd, k0:k0 + ks, :])
                    row.append(t)
                WV_t.append(row)
            Wq_t, Wk_t = [], []
            for kb_i, (k0, ks) in enumerate(KBLK):
                tq = wp.tile([ks, C8], BF16, name=f"Wq_{kb_i}",
                             tag=f"Wq_{kb_i}")
                nc.sync.dma_start(out=tq[:], in_=Wq_d[k0:k0 + ks, :])
                Wq_t.append(tq)
                tk = wp.tile([ks, C8], BF16, name=f"Wk_{kb_i}",
                             tag=f"Wk_{kb_i}")
                nc.sync.dma_start(out=tk[:], in_=Wk_d[k0:k0 + ks, :])
                Wk_t.append(tk)
            ones_t = wp.tile([C8, 128], BF16, name="ones_t", tag="ones_t")
            nc.sync.dma_start(out=ones_t[:], in_=ones_d[:])
            vbias_t = []
            for bl in range(2):
                t = wp.tile([128, 1], F32, name=f"vbias_{bl}",
                            tag=f"vbias_{bl}")
                nc.sync.dma_start(
                    out=t[:], in_=vbias_d[bl * 128:(bl + 1) * 128].unsqueeze(1))
                vbias_t.append(t)
            qb_t = wp.tile([C8, 1], F32, name="qb_t", tag="qb_t")
            nc.sync.dma_start(out=qb_t[:], in_=qb_d[:].unsqueeze(1))
            kb_t = wp.tile([C8, 1], F32, name="kb_t", tag="kb_t")
            nc.sync.dma_start(out=kb_t[:], in_=kb_d[:].unsqueeze(1))
            for g in range(2, NB):
                load_band(g)
            WB_t = []
            for cc in range(16):
                row = []
                for kb_i, (k0, ks) in enumerate(KBLK):
                    t = wp.tile([ks, CO], BF16, name=f"WB_{cc}_{kb_i}",
                                tag=f"WB_{cc}_{kb_i}")
                    nc.sync.dma_start(out=t[:], in_=WB_d[cc, k0:k0 + ks, :])
                    row.append(t)
                WB_t.append(row)

            # ================= Part A emitter (per group) ===============
            ot_tiles = {}

            def part_a(g, mb_list=(0, 1)):
                """Fused-conv Part A for row group g: psum accumulate then
                ACT-engine copy into the group output tile."""
                for mb_i in mb_list:
                    m0, ms = MBLK[mb_i]
                    ot = gop.tile([ms, 16 * W], F32,
                                  name=f"ot{g % 2}_{mb_i}",
                                  tag=f"ot{g % 2}_{mb_i}")
                    ot_tiles[(g, mb_i)] = ot
                    for n in range(4):
                        pa_ = psa.tile([ms, 4 * W], F32, name="psAt",
                                       tag="psAt")
                        first = True
                        for dy in range(3):
                            for dx in range(3):
                                for kb_i in range(2):
                                    bv = bands[g][kb_i][:].rearrange(
                                        "p (r c) -> p r c", c=HB)
                                    rhs = bv[:, 4 * n + dy:4 * n + dy + 4,
                                             dx:dx + W]
                                    nc.tensor.matmul(
                                        pa_[:],
                                        WA_t[dy * 3 + dx][kb_i][:,
                                                                m0:m0 + ms],
                                        rhs,
                                        start=first,
                                        stop=(dy == 2 and dx == 2
                                              and kb_i == 1))
                                    first = False
                        nc.scalar.copy(
                            out=ot[:, n * 4 * W:(n + 1) * 4 * W],
                            in_=pa_[:])

            # ================= Part B emitter (per group) ===============
            def part_b(psb, tp_t, g):
                """Part B phases for group g: scatter-add (+bias) into the
                group output tile, then DMA the tile out."""
                for mb_i, (m0, ms) in enumerate(MBLK):
                    ot = ot_tiles.pop((g, mb_i))
                    sv = ot[:].rearrange(
                        "p (h two w pw2) -> p h two w pw2",
                        two=2, w=WP, pw2=2)
                    for ph in range(2):
                        ro = _tap_offsets(ph)
                        for pw in range(2):
                            co = _tap_offsets(pw)
                            pb_ = psb.tile([ms, 8 * WP], F32, name="psBt",
                                           tag="psBt")
                            first = True
                            for i in range(2):
                                for j in range(2):
                                    cc = ((ph * 2 + pw) * 2 + i) * 2 + j
                                    for kb_i in range(2):
                                        tv = tp_t[kb_i][:].rearrange(
                                            "p (r c) -> p r c", c=TPW)
                                        rhs = tv[:,
                                                 8 * g + 1 + ro[i]:
                                                 8 * g + 9 + ro[i],
                                                 1 + co[j]:
                                                 1 + co[j] + WP]
                                        nc.tensor.matmul(
                                            pb_[:],
                                            WB_t[cc][kb_i][:, m0:m0 + ms],
                                            rhs,
                                            start=first,
                                            stop=(i == 1 and j == 1
                                                  and kb_i == 1))
                                        first = False
                            osl = sv[:, :, ph, :, pw]
                            nc.vector.scalar_tensor_tensor(
                                out=osl,
                                in0=pb_[:].rearrange("p (h w) -> p h w",
                                                     w=WP),
                                scalar=fb_t[mb_i][:], in1=osl,
                                op0=AluOpType.add, op1=AluOpType.add)
                    nc.sync.dma_start(
                        out=out_d[16 + m0:16 + m0 + ms,
                                  16 * g:16 * (g + 1), :],
                        in_=ot[:].rearrange("p (r c) -> p r c", c=W))

            # ===================== PE schedule ==========================
            part_a(0)

            tp_t = []
            with tc.tile_pool(name="psp", bufs=1, space="PSUM") as psp:
                # ---- vsum ----
                vsum_t = []
                for mb in range(2):
                    ps = psp.tile([128, W], F32, name="psS", tag="psS")
                    first = True
                    for sd in range(9):
                        src, dx = divmod(sd, 3)
                        for kb_i in range(2):
                            nc.tensor.matmul(
                                ps[:],
                                WV_t[sd][kb_i][:, mb * 128:(mb + 1) * 128],
                                X3b[kb_i][:, src, dx:dx + W],
                                start=first, stop=(sd == 8 and kb_i == 1))
                            first = False
                    ssb = prp.tile([128, W], F32, name=f"ssb_{mb}",
                                   tag=f"ssb_{mb}")
                    nc.scalar.activation(out=ssb[:], in_=ps[:],
                                         func=AF.Copy, scale=0.25)
                    se = ssb[:].rearrange("p (w two) -> p w two", two=2)
                    vs = prp.tile([128, WP], F32, name=f"vsum_{mb}",
                                  tag=f"vsum_{mb}")
                    nc.vector.scalar_tensor_tensor(
                        out=vs[:], in0=se[:, :, 0],
                        scalar=vbias_t[mb][:], in1=se[:, :, 1],
                        op0=AluOpType.add, op1=AluOpType.add)
                    vsum_t.append(vs)

                # ---- k ----
                psk = psp.tile([C8, WP], F32, name="psK", tag="psK")
                nc.tensor.matmul(psk[:], Wk_t[0][:], P2b[0][:],
                                 start=True, stop=False)
                nc.tensor.matmul(psk[:], Wk_t[1][:], P2b[1][:],
                                 start=False, stop=True)
                k_t = prp.tile([C8, WP], F32, name="k_t", tag="k_t")
                nc.vector.tensor_scalar_add(out=k_t[:], in0=psk[:],
                                            scalar1=kb_t[:])

                # ---- q (all groups; kb-outer for weight reuse) ----
                qk_t = prp.tile([C8, HP * WP], BF16, name="qk_t",
                                tag="qk_t")
                qkv = qk_t[:].rearrange("p (h w) -> p h w", w=WP)
                for g in range(NB):
                    psq = psp.tile([C8, 8 * WP], F32, name="psQ",
                                   tag="psQ", bufs=2)
                    for kb_i in range(2):
                        bv = bands[g][kb_i][:].rearrange(
                            "p (r c) -> p r c", c=HB)
                        for i in range(2):
                            for j in range(2):
                                rhs = bv[:, 1 + i:17 + i:2, 1 + j:129 + j:2]
                                nc.tensor.matmul(
                                    psq[:], Wq_t[kb_i][:], rhs,
                                    start=(kb_i == 0 and i == 0 and j == 0),
                                    stop=(kb_i == 1 and i == 1 and j == 1))
                    qtmp = prp.tile([C8, 8 * WP], F32, name="qtmp",
                                    tag="qtmp", bufs=2)
                    nc.vector.tensor_scalar_add(out=qtmp[:], in0=psq[:],
                                                scalar1=qb_t[:])
                    kv = k_t[:].unsqueeze(1).broadcast_to([C8, 8, WP])
                    nc.vector.tensor_tensor(
                        out=qkv[:, g * 8:(g + 1) * 8, :],
                        in0=qtmp[:].rearrange("p (h w) -> p h w", w=WP),
                        in1=kv, op=AluOpType.mult)

                part_a(1, mb_list=(0,))

                # ---- e (replicated) + column norms (Pool engine) ----
                e_t = prp.tile([128, HP * WP], BF16, name="e_t", tag="e_t")
                n2 = prp.tile([128, WP], F32, name="n2", tag="n2")
                for chn in range(8):
                    nsl = slice(chn * 512, (chn + 1) * 512)
                    pse = psp.tile([128, 512], F32, name="psE", tag="psE")
                    nc.tensor.matmul(pse[:], ones_t[:], qk_t[:, nsl],
                                     start=True, stop=True)
                    nc.vector.tensor_copy(out=e_t[:, nsl], in_=pse[:])
                    scr = prp.tile([128, 8 * WP], F32, name="scr",
                                   tag="scr", bufs=2)
                    esl = e_t[:, chn * 8 * WP:(chn + 1) * 8 * WP]
                    nc.gpsimd.tensor_tensor(out=scr[:], in0=esl, in1=esl,
                                            op=AluOpType.mult)
                    lvl, sz = scr, 4 * WP
                    for d in range(3):
                        nxt = prp.tile([128, sz], F32, name=f"nt{d}",
                                       tag=f"nt{d}", bufs=2)
                        nc.gpsimd.tensor_tensor(
                            out=nxt[:], in0=lvl[:, 0:sz],
                            in1=lvl[:, sz:2 * sz], op=AluOpType.add)
                        lvl, sz = nxt, sz // 2
                    if chn == 0:
                        nc.gpsimd.tensor_copy(out=n2[:], in_=lvl[:])
                    else:
                        nc.gpsimd.tensor_add(out=n2[:], in0=n2[:],
                                             in1=lvl[:])
                nrm = prp.tile([128, WP], F32, name="nrm", tag="nrm")
                nc.scalar.sqrt(out=nrm[:], in_=n2[:])
                rinv = prp.tile([128, WP], F32, name="rinv", tag="rinv")
                nc.vector.reciprocal(out=rinv[:], in_=nrm[:])

                part_a(1, mb_list=(1,))

                # ---- Tp_pad = (vsum * rinv) x e ----
                for bl in range(2):
                    vs2 = prp.tile([128, WP], F32, name=f"vs2_{bl}",
                                   tag=f"vs2_{bl}")
                    nc.vector.tensor_tensor(
                        out=vs2[:], in0=vsum_t[bl][:], in1=rinv[:],
                        op=AluOpType.mult)
                    tp = stp.tile([128, (HP + 2) * TPW], BF16,
                                  name=f"tp_{bl}", tag=f"tp_{bl}")
                    tp_t.append(tp)
                    tv = tp[:].rearrange("p (r c) -> p r c", c=TPW)
                    nc.gpsimd.memset(tv[:, 0:1, :], 0.0)
                    nc.gpsimd.memset(tv[:, HP + 1:HP + 2, :], 0.0)
                    nc.gpsimd.memset(tv[:, :, 0:1], 0.0)
                    nc.gpsimd.memset(tv[:, :, TPW - 1:TPW], 0.0)
                    ev = e_t[:].rearrange("p (h w) -> p h w", w=WP)
                    v2 = vs2[:].unsqueeze(1).broadcast_to([128, HP, WP])
                    nc.vector.tensor_tensor(
                        out=tv[:, 1:1 + HP, 1:1 + WP], in0=v2, in1=ev,
                        op=AluOpType.mult)

            # prefix psum pool closed; Part B reuses its banks (ordered
            # transitively: B's matmuls wait on Tp, which sits behind all
            # prefix psum readers on the vector/ACT/Pool queues).
            with tc.tile_pool(name="psb", bufs=3, space="PSUM") as psb:
                part_b(psb, tp_t, 0)
                part_b(psb, tp_t, 1)
                for g in range(2, NB):
                    part_a(g)
                    part_b(psb, tp_t, g)

            # out[:16] = bev[:16] straight through, DRAM->DRAM (queued
            # last: no consumer inside the kernel)
            nc.sync.dma_start(out=out_d[0:16], in_=b16_d[:])
    return nc


def _dedup_ldweights(nc):
    """Tile splits every matmul into a standalone InstLdweights plus a
    non-self-loading InstMatmult. Consecutive matmuls reusing the same
    stationary therefore emit redundant PE-array loads. Delete an
    InstLdweights when the previous one on the PE stream loaded the
    identical weights AP and nothing in between clobbered the array.
    Ldweights carry no semaphore updates; any vestigial waits are folded
    into the next matmul (split later by `_legalize_waits` if needed)."""
    n_drop = 0
    PE = mybir.EngineType.PE
    for fn in nc.m.functions:
        for bb in fn.blocks:
            out = []
            last_key = None
            pending_waits = []
            for ins in bb.instructions:
                if isinstance(ins, mybir.InstLdweights):
                    key = (str(ins.ins[0]), str(ins.tile_position),
                           str(ins.tile_size), str(ins.perf_mode),
                           str(ins.is_transpose))
                    if key == last_key:
                        si = ins.sync_info
                        if si is not None and si.on_update:
                            out.append(ins)   # cannot drop: has updates
                            continue
                        if si is not None and si.on_wait:
                            pending_waits.extend(si.on_wait)
                        n_drop += 1
                        continue
                    last_key = key
                    out.append(ins)
                elif isinstance(ins, mybir.InstMatmult):
                    if ins.is_transpose:
                        last_key = None
                    if pending_waits:
                        si = ins.sync_info
                        w = list(si.on_wait) if si else []
                        u = list(si.on_update) if si else []
                        ins.sync_info = mybir.SyncInfo(
                            on_wait=w + pending_waits, on_update=u)
                        pending_waits = []
                    out.append(ins)
                else:
                    if (getattr(ins, "engine", None) == PE
                            and not isinstance(ins,
                                               mybir.InstEventSemaphore)):
                        last_key = None
                    out.append(ins)
            assert not pending_waits, "dangling ldweights waits after dedup"
            bb.instructions[:] = out
    return n_drop


def _legalize_waits(nc):
    """This toolchain's codegen accepts at most ONE semaphore wait per
    instruction (the TPB `events` field has a single wait slot). Tile's
    wait assignment can attach several. Hoist all but one wait onto
    standalone EventSemaphore instructions placed immediately before the
    owner on the same engine stream - strictly stronger synchronization,
    so always safe."""
    n_split = 0
    for fn in nc.m.functions:
        for bb in fn.blocks:
            out = []
            for ins in bb.instructions:
                si = ins.sync_info
                if si is not None and len(si.on_wait) > 1:
                    extra = list(si.on_wait[:-1])
                    keep = si.on_wait[-1]
                    for idx, wt in enumerate(extra):
                        ev = mybir.InstEventSemaphore(
                            name=f"{ins.name}_hw{idx}",
                            engine=ins.engine,
                            sync_info=mybir.SyncInfo(on_wait=[wt],
                                                     on_update=[]),
                        )
                        out.append(ev)
                        n_split += 1
                    ins.sync_info = mybir.SyncInfo(
                        on_wait=[keep], on_update=list(si.on_update))
                out.append(ins)
            bb.instructions[:] = out
    return n_split


_NC_CACHE = None


def kernel(**inputs):
    global _NC_CACHE
    in_maps = _prep_inputs(inputs)
    if _NC_CACHE is None:
        _NC_CACHE = _build_module()
        _dedup_ldweights(_NC_CACHE)
        _legalize_waits(_NC_CACHE)
    res = run_bass_kernel_spmd(_NC_CACHE, in_maps, list(range(NCORES)))
    out = np.stack([res.results[b]["out"] for b in range(NCORES)], axis=0)
    return out.astype(np.float32)
